# revision 1
# baseline (speedup 1.0000x reference)
"""GroupDense kernel for Trainium2 (8 NeuronCores, SPMD data-parallel over batch).

y[b,s,g*64+v] = relu(sum_u x[b,s,g*64+u] * w[g,u,v])
x: [8, 2048, 4096] fp32, w: [64, 64, 64] fp32.

Per-core: core i processes batch i ([2048, 4096], 32 MB in / 32 MB out).
Weights are packed host-side into 32 block-diagonal [128,128] bf16 tiles
(two 64x64 groups per tile) so each matmul contracts over K=128 with full
PE utilization. x must be transposed on-chip (contraction dim on
partitions): PE transpose (fp32) -> PSUM -> DVE copy+cast to bf16 ->
SBUF, then bf16 matmul (1 cyc/row vs fp32's 4), ReLU on ACT.

The kernel is HBM-bound (64 MB I/O per core; the 16 SDMA engines
sustain ~427 GB/s => ~161 us data floor). All I/O rides the SP HWDGE
ring (nc.sync) as one FIFO, interleaved in(t+PREF), out(t): reads run
PREF tiles ahead so compute always has data, and each out chunk
reaches the FIFO head long after its ReLU finished. YBUFS >= PREF+1
lets compute run ahead of write drains through the drain phase, so the
pipe never starves, start to finish. Keep DMA chunks at 1 MB: per-DMA
ring overhead (~0.6 us) only hides under a >=1 MB drain time.
"""

import numpy as np

import concourse.bass as bass
import concourse.mybir as mybir
import concourse.tile as tile
from concourse import bacc
from concourse.bass import ds, ts
from concourse.bass_utils import run_bass_kernel_spmd
from concourse.masks import make_identity

B, S, C = 8, 2048, 4096
U = 64
G = C // U  # 64 groups
NCORES = 8
TOK = (B * S) // NCORES  # 2048 tokens per core
P = 128
CB = C // P   # 32 channel blocks (2 groups each)
TT = TOK // P  # 16 token tiles

F32 = mybir.dt.float32
F32R = mybir.dt.float32r
BF16 = mybir.dt.bfloat16

_cached_nc = None
_cfg = {}


def _build():
    global _cached_nc
    if _cached_nc is not None:
        return _cached_nc

    nc = bacc.Bacc("TRN2", target_bir_lowering=False)

    x_d = nc.dram_tensor("x", [TOK, C], F32, kind="ExternalInput")
    # host pre-packs weights partition-major bf16 block-diagonal pairs.
    w_d = nc.dram_tensor("w2", [P, CB * P], BF16, kind="ExternalInput")
    y_d = nc.dram_tensor("y", [TOK, C], F32, kind="ExternalOutput")

    OCT = 8  # channel blocks per unit: 2 PSUM banks per psum tile
    UNITS = CB // OCT

    XBUFS = _cfg.get("xbufs", 6)
    YBUFS = _cfg.get("ybufs", 5)
    IN_CHUNKS = _cfg.get("in_chunks", 2)
    OUT_EVERY = _cfg.get("out_every", 2)  # store y every N units
    PREF = _cfg.get("pref", 4)  # input tiles prefetched ahead of compute
    PARK = _cfg.get("park", 0)  # early tiles whose writes are deferred to the end

    with tile.TileContext(nc) as tc:
        with (
            tc.tile_pool(name="consts", bufs=1) as consts,
            tc.tile_pool(name="wpool", bufs=1) as wpool,
            tc.tile_pool(name="xpool", bufs=XBUFS) as xpool,
            tc.tile_pool(name="xtpool", bufs=3) as xtpool,
            tc.tile_pool(name="ypool", bufs=YBUFS) as ypool,
            tc.tile_pool(name="ypark", bufs=max(PARK, 1)) as ypark,
            tc.tile_pool(name="psT", bufs=2, space="PSUM") as psT,
            tc.tile_pool(name="psY", bufs=2, space="PSUM") as psY,
        ):
            ident = consts.tile([P, P], F32)
            make_identity(nc, ident[:])

            # weights ride the ACT HWDGE ring; first x chunk rides SP's,
            # so both start immediately and concurrently.
            w_s = wpool.tile([P, CB, P], BF16)
            nc.scalar.dma_start(w_s[:], w_d[:, :])

            CHUNK = OCT * P  # 1024 channels per unit
            xtiles = {}

            def issue_in(tt):
                x_t = xpool.tile([P, C], F32)
                xtiles[tt] = x_t
                icw = C // IN_CHUNKS
                for ic in range(IN_CHUNKS):
                    nc.sync.dma_start(
                        x_t[:, ds(ic * icw, icw)],
                        x_d[ts(tt, P), ds(ic * icw, icw)],
                    )

            parked = {}

            def flush_out(tt):
                y_t = parked.pop(tt)
                for oc in range(UNITS // OUT_EVERY):
                    ow = OUT_EVERY * CHUNK
                    nc.sync.dma_start(
                        y_d[ts(tt, P), ds(oc * ow, ow)],
                        y_t[:, ds(oc * ow, ow)],
                    )

            def compute_out(tt, defer=False):
                x_t = xtiles.pop(tt)
                if defer:
                    y_t = ypark.tile([P, C], F32)
                    parked[tt] = y_t
                else:
                    y_t = ypool.tile([P, C], F32)
                for q in range(UNITS):
                    # NOTE: f32r transposes (1.5 cyc/row) fail walrus BIR
                    # compile on this toolchain; keep plain fp32.
                    pT = psT.tile([P, OCT, P], F32)
                    for j in range(OCT):
                        cb = OCT * q + j
                        nc.tensor.transpose(
                            pT[:, j, :], x_t[:, ts(cb, P)], ident[:]
                        )
                    xT = xtpool.tile([P, OCT, P], BF16)
                    nc.vector.tensor_copy(xT[:], pT[:])

                    pY = psY.tile([P, OCT, P], F32)
                    for j in range(OCT):
                        cb = OCT * q + j
                        nc.tensor.matmul(
                            pY[:, j, :], xT[:, j, :], w_s[:, cb, :],
                            start=True, stop=True,
                        )
                    nc.scalar.activation(
                        y_t[:, ds(q * CHUNK, CHUNK)], pY[:],
                        mybir.ActivationFunctionType.Relu,
                    )
                    if not defer and (q + 1) % OUT_EVERY == 0:
                        ow = OUT_EVERY * CHUNK
                        oc = (q + 1) // OUT_EVERY - 1
                        nc.sync.dma_start(
                            y_d[ts(tt, P), ds(oc * ow, ow)],
                            y_t[:, ds(oc * ow, ow)],
                        )

            # One FIFO on the SP ring: reads run ahead of writes, and the
            # first PARK tiles' writes are deferred to the very end so the
            # drain phase streams pre-computed outputs with no dependency
            # stalls. FIFO: in0..in(PREF-1) | (in(t), out(t-PREF+PARK)) |
            # trailing outs | parked outs.
            for tt in range(PREF):
                issue_in(tt)
            for tt in range(PARK):
                compute_out(tt, defer=True)
            for tt in range(PREF, TT):
                issue_in(tt)
                compute_out(tt - PREF + PARK)
            for tt in range(TT - PREF + PARK, TT):
                compute_out(tt)
            for tt in range(PARK):
                flush_out(tt)

    nc.compile()
    _cached_nc = nc
    return nc


def _pack_weights(kern):
    w2 = np.zeros((CB, P, P), dtype=np.float32)
    w2[:, :U, :U] = kern[0::2]
    w2[:, U:, U:] = kern[1::2]
    w2 = np.ascontiguousarray(w2.transpose(1, 0, 2).reshape(P, CB * P))
    import ml_dtypes

    return w2.astype(ml_dtypes.bfloat16)


def kernel(x, kernel):
    x = np.ascontiguousarray(x, dtype=np.float32)
    w2 = _pack_weights(np.asarray(kernel, dtype=np.float32))

    nc = _build()
    in_maps = [
        {"x": np.ascontiguousarray(x[i].reshape(TOK, C)), "w2": w2}
        for i in range(NCORES)
    ]
    res = run_bass_kernel_spmd(nc, in_maps, list(range(NCORES)))
    y = np.stack([res.results[i]["y"] for i in range(NCORES)], axis=0)
    return y.reshape(B, S, C)



# revision 2
# speedup vs baseline: 1.9320x; 1.9320x over previous
"""GroupDense kernel for Trainium2 (8 NeuronCores, SPMD data-parallel over batch).

y[b,s,g*64+v] = relu(sum_u x[b,s,g*64+u] * w[g,u,v])
x: [8, 2048, 4096] fp32, w: [64, 64, 64] fp32.

Per-core: core i processes batch i (2048 tokens x 4096 channels).

HBM traffic is the roofline, so both directions ride bf16 and the
layout is chosen so NO on-chip transpose is needed:
- host packs x transposed + bf16: xt[p, cb, t] = x[t, cb*128+p]
  ([128, 32*2048] bf16, 16 MB/core). Contraction dim (channel) is
  already on partitions.
- weights are packed block-diagonal bf16 [128, 32*128] (two 64x64
  groups per 128x128 tile) and used as the STATIONARY matmul operand:
  one LDWEIGHTS per (cb, psum-chunk) instead of per x-tile.
- matmul(out=yT, lhsT=w_cb, rhs=xt_cb) -> yT[v, t] in PSUM (fp32).
- ReLU + cast to bf16 PSUM->SBUF, alternating ACT/DVE so neither
  engine is the bottleneck, then DMA out yT [128, 32*2048] bf16
  (16 MB/core). Host un-transposes + casts back to fp32.

Per-core HBM: 32 MB + 1 MB weights => ~92 us floor at ~358 GB/s.
All x/y I/O rides the SP HWDGE ring (nc.sync) as one FIFO with reads
running PREF units ahead of writes (weights ride the ACT ring so both
start concurrently). 1 MB DMA chunks keep ring overhead (~0.6us)
hidden under drain time.
"""

import numpy as np

import concourse.bass as bass
import concourse.mybir as mybir
import concourse.tile as tile
from concourse import bacc
from concourse.bass import ds, ts
from concourse.bass_utils import run_bass_kernel_spmd

B, S, C = 8, 2048, 4096
U = 64
G = C // U  # 64 groups
NCORES = 8
TOK = (B * S) // NCORES  # 2048 tokens per core
P = 128
CB = C // P   # 32 channel blocks (2 groups each)

F32 = mybir.dt.float32
BF16 = mybir.dt.bfloat16

_cached_nc = None
_cfg = {}


def _build():
    global _cached_nc
    if _cached_nc is not None:
        return _cached_nc

    nc = bacc.Bacc("TRN2", target_bir_lowering=False)

    # host pre-packs x channel-major bf16: row p holds x[:, cb*128+p]
    # for all cb, tokens contiguous per (p, cb).
    xt_d = nc.dram_tensor("xt", [P, CB * TOK], BF16, kind="ExternalInput")
    # host pre-packs weights partition-major bf16 block-diagonal pairs.
    w_d = nc.dram_tensor("w2", [P, CB * P], BF16, kind="ExternalInput")
    y_d = nc.dram_tensor("y", [P, CB * TOK], BF16, kind="ExternalOutput")

    OCB = _cfg.get("ocb", 2)     # channel blocks per unit (1 MB in, 1 MB out)
    QN = CB // OCB               # 16 units
    NT = TOK // 512              # 4 psum chunks of 512 tokens per cb

    XBUFS = _cfg.get("xbufs", 6)
    YBUFS = _cfg.get("ybufs", 5)
    PREF = _cfg.get("pref", 4)   # input units prefetched ahead of compute
    PARK = _cfg.get("park", 0)   # early units whose writes are deferred to the end

    with tile.TileContext(nc) as tc:
        with (
            tc.tile_pool(name="wpool", bufs=1) as wpool,
            tc.tile_pool(name="xpool", bufs=XBUFS) as xpool,
            tc.tile_pool(name="ypool", bufs=YBUFS) as ypool,
            tc.tile_pool(name="ypark", bufs=max(PARK, 1)) as ypark,
            tc.tile_pool(name="psY", bufs=2, space="PSUM") as psY,
        ):
            # weights ride the ACT HWDGE ring; first x chunk rides SP's,
            # so both start immediately and concurrently.
            w_s = wpool.tile([P, CB, P], BF16)
            nc.scalar.dma_start(w_s[:], w_d[:, :])

            xtiles = {}

            def issue_in(q):
                x_t = xpool.tile([P, OCB, TOK], BF16)
                xtiles[q] = x_t
                nc.sync.dma_start(
                    x_t[:], xt_d[:, ds(q * OCB * TOK, OCB * TOK)]
                )

            parked = {}

            def flush_out(q):
                y_t = parked.pop(q)
                nc.sync.dma_start(
                    y_d[:, ds(q * OCB * TOK, OCB * TOK)], y_t[:]
                )

            def compute_out(q, defer=False):
                x_t = xtiles.pop(q)
                if defer:
                    y_t = ypark.tile([P, OCB, TOK], BF16)
                    parked[q] = y_t
                else:
                    y_t = ypool.tile([P, OCB, TOK], BF16)
                for j in range(OCB):
                    cb = q * OCB + j
                    pY = psY.tile([P, NT, 512], F32)
                    for n in range(NT):
                        nc.tensor.matmul(
                            pY[:, n, :], w_s[:, cb, :],
                            x_t[:, j, ts(n, 512)],
                            start=True, stop=True,
                        )
                    # alternate ReLU+cast between ACT and DVE
                    if (q * OCB + j) % 2 == 0:
                        nc.scalar.activation(
                            y_t[:, j, :], pY[:],
                            mybir.ActivationFunctionType.Relu,
                        )
                    else:
                        nc.vector.tensor_scalar_max(y_t[:, j, :], pY[:], 0.0)
                if not defer:
                    nc.sync.dma_start(
                        y_d[:, ds(q * OCB * TOK, OCB * TOK)], y_t[:]
                    )

            # One FIFO on the SP ring: reads run ahead of writes; the first
            # PARK units' writes are deferred to the very end so the drain
            # phase streams pre-computed outputs with no dependency stalls.
            for q in range(PREF):
                issue_in(q)
            for q in range(PARK):
                compute_out(q, defer=True)
            for q in range(PREF, QN):
                issue_in(q)
                compute_out(q - PREF + PARK)
            for q in range(QN - PREF + PARK, QN):
                compute_out(q)
            for q in range(PARK):
                flush_out(q)

    nc.compile()
    _cached_nc = nc
    return nc


def _pack_weights(kern):
    w2 = np.zeros((CB, P, P), dtype=np.float32)
    w2[:, :U, :U] = kern[0::2]
    w2[:, U:, U:] = kern[1::2]
    w2 = np.ascontiguousarray(w2.transpose(1, 0, 2).reshape(P, CB * P))
    import ml_dtypes

    return w2.astype(ml_dtypes.bfloat16)


def _pack_x(xi):
    """[TOK, C] fp32 -> [P, CB*TOK] bf16 with xt[p, cb*TOK+t] = x[t, cb*128+p]."""
    import ml_dtypes

    xt = xi.reshape(TOK, CB, P).astype(ml_dtypes.bfloat16)
    return np.ascontiguousarray(xt.transpose(2, 1, 0)).reshape(P, CB * TOK)


def _unpack_y(yi):
    """[P, CB*TOK] bf16 -> [TOK, C] fp32 inverse of _pack_x."""
    y = yi.reshape(P, CB, TOK).transpose(2, 1, 0).reshape(TOK, C)
    return y.astype(np.float32)


def _make_in_maps(x, kern):
    x = np.asarray(x, dtype=np.float32)
    w2 = _pack_weights(np.asarray(kern, dtype=np.float32))
    return [
        {"xt": _pack_x(x[i].reshape(TOK, C)), "w2": w2} for i in range(NCORES)
    ]


def kernel(x, kernel):
    nc = _build()
    in_maps = _make_in_maps(x, kernel)
    res = run_bass_kernel_spmd(nc, in_maps, list(range(NCORES)))
    y = np.stack([_unpack_y(res.results[i]["y"]) for i in range(NCORES)], axis=0)
    return y.reshape(B, S, C)
